# revision 1
# baseline (speedup 1.0000x reference)
"""CAM (channel attention) module kernel for Trainium2 (Bass/Tile).

Reference computation (per batch b):
    energy  = x_b @ x_b.T                      # [C, C], contraction over N
    att     = softmax(rowmax(energy) - energy) # row-wise over last axis
    out     = att @ x_b                        # [C, N]
    y_b     = gamma * out + x_b

Sharding: data-parallel over B across 8 NeuronCores (B=32 -> 4 per core),
gamma replicated, full CxC attention per core.

Identity used: softmax(rowmax(E) - E)[i,j] = exp(mn[i] - E[i,j]) / Z[i]
with mn[i] = min_j E[i,j], Z[i] = sum_j exp(mn[i] - E[i,j])  (shift
invariance of softmax; exact).

Layouts per batch (P=128 partitions):
    X   [P, CO, N]  c-natural  (c = co*P + p)           -- DMA from DRAM
    X16 [P, CO, N]  bf16 copy (matmul-2 moving operand)  -- GpSimd cast
    xT  [P, C]      per k-chunk, n on partitions         -- PE transpose, f32r
    E   [P, CO, C]  PSUM, i on partitions, j on free     -- matmul 1 (f32r)
    t   [P, CO, C]  SBUF f32, exp(mn - E), Z fused       -- ScalarE activation
    tT  [P, CO, C]  SBUF bf16, j on partitions           -- PE transpose
    out chunk [P, 512] = (tT.T @ X16) * (gamma/Z[i]) + X -- matmul 2 + DVE

`reps` wraps the whole body in a hardware loop (identical work each
iteration, static addressing) -- used only for timing runs.
"""

import contextlib

import numpy as np

P = 128

_CACHE = {}


DEFAULT_OPTS = dict(
    xt_bufs=12,     # xT k-chunk SBUF tiles
    psx_bufs=2,     # PSUM banks for x-transpose staging
    acc_bufs=2,     # PSUM banks shared by t-transpose + matmul-2
    o_bufs=6,       # output staging tiles
    evac_split=False,  # True: ACT scales PSUM->SBUF, DVE adds residual
    alt_acc=False,  # alternate two bufs=1 tags for ps2/tt (force bank ping-pong)
    alt_psx=False,  # alternate two bufs=1 tags for x-transpose staging
    sw_pipe1=True,  # emit transposes one k-chunk ahead of matmul-1 groups
    tt_sym=False,   # tT from E symmetry (DVE sub + ACT exp), no PE t-transposes
    timing_io=False,  # x/y internal DRAM (no host transfer) -- timing runs only
)


def _build(Bs, C, N, use_f32r=True, reps=1, **opts):
    import concourse.bass as bass  # noqa: F401
    import concourse.tile as tile
    import concourse.mybir as mybir
    from concourse import bacc
    from concourse.masks import make_identity

    o = dict(DEFAULT_OPTS)
    o.update(opts)

    F32 = mybir.dt.float32
    BF16 = mybir.dt.bfloat16
    MMDT = mybir.dt.float32r if use_f32r else mybir.dt.bfloat16
    AF = mybir.ActivationFunctionType
    ALU = mybir.AluOpType
    AX = mybir.AxisListType

    assert C == 4 * P and N % 512 == 0
    CO = C // P          # i/j chunks of 128
    KC = N // P          # n chunks of 128 (contraction for energy)
    NF = N // 512        # n chunks of 512 (matmul-2 free dim)

    nc = bacc.Bacc(None, target_bir_lowering=False, debug=False)
    if o["timing_io"]:
        x_in = nc.dram_tensor("x_int", [Bs, C, N], F32)
        g_in = nc.dram_tensor("gamma", [1], F32, kind="ExternalInput")
        y_out = nc.dram_tensor("y_int", [Bs, C, N], F32)
        yy_out = nc.dram_tensor("yy", [1, 1], F32, kind="ExternalOutput")
    else:
        x_in = nc.dram_tensor("x", [Bs, C, N], F32, kind="ExternalInput")
        g_in = nc.dram_tensor("gamma", [1], F32, kind="ExternalInput")
        y_out = nc.dram_tensor("y", [Bs, C, N], F32, kind="ExternalOutput")
        yy_out = None

    with tile.TileContext(nc) as tc:
        with (
            tc.tile_pool(name="consts", bufs=1) as consts,
            tc.tile_pool(name="xpool", bufs=2) as xpool,
            tc.tile_pool(name="x16pool", bufs=1) as x16pool,
            tc.tile_pool(name="xtp", bufs=o["xt_bufs"]) as xtp,
            tc.tile_pool(name="tpool", bufs=1) as tpool,
            tc.tile_pool(name="ttpool", bufs=1 if o["tt_sym"] else 2) as ttpool,
            tc.tile_pool(name="opool", bufs=o["o_bufs"]) as opool,
            tc.tile_pool(name="stats", bufs=2) as stats,
            tc.tile_pool(name="pe", bufs=1, space="PSUM") as psum_e,
            tc.tile_pool(name="pxt", bufs=o["psx_bufs"], space="PSUM") as psum_xt,
            tc.tile_pool(name="pacc", bufs=o["acc_bufs"], space="PSUM") as psum_acc,
        ):
            ident = consts.tile([P, P], F32)
            make_identity(nc, ident)
            g_sb = consts.tile([1, 1], F32)
            nc.sync.dma_start(g_sb[:, :], g_in[:].rearrange("(a b) -> a b", a=1))
            g_col = consts.tile([P, 1], F32)
            nc.gpsimd.partition_broadcast(g_col[:, :], g_sb[:1, :1])

            if o["timing_io"]:
                # zero-fill the internal x so the body sees finite data
                zt = opool.tile([P, 512], F32, tag="o", name="zt")
                nc.gpsimd.memset(zt[:, :], 0.0)
                for zb in range(Bs):
                    zx = x_in[zb].rearrange("(co p) n -> p co n", p=P)
                    for zco in range(CO):
                        for znf in range(NF):
                            nc.sync.dma_start(
                                zx[:, zco, znf * 512:(znf + 1) * 512],
                                zt[:, :],
                            )

            loop_ctx = (
                tc.For_i(0, reps, 1) if reps > 1 else contextlib.nullcontext()
            )
            with loop_ctx:
                for b in range(Bs):
                    x_b = x_in[b].rearrange("(co p) n -> p co n", p=P)
                    y_b = y_out[b].rearrange("(co p) n -> p co n", p=P)

                    X = xpool.tile([P, CO, N], F32, tag="X")
                    # first 128 columns land alone so transposes start early
                    nc.sync.dma_start(X[:, :, 0:P], x_b[:, :, 0:P])
                    nc.sync.dma_start(X[:, :, P:512], x_b[:, :, P:512])
                    for nf in range(1, NF):
                        s = slice(nf * 512, (nf + 1) * 512)
                        nc.sync.dma_start(X[:, :, s], x_b[:, :, s])

                    # bf16 copy of x for matmul-2's moving operand
                    X16 = x16pool.tile([P, CO, N], BF16, tag="X16")
                    for co in range(CO):
                        nc.gpsimd.tensor_copy(X16[:, co, :], X[:, co, :])

                    # ---- energy = x @ x.T (contraction over n on partitions)
                    # E is symmetric: compute upper-triangular blocks only
                    # (row ic covers columns >= ic*P), mirror the rest after.
                    E = psum_e.tile([P, CO, C], F32, tag="E")

                    def emit_trans(kc):
                        ks = slice(kc * P, (kc + 1) * P)
                        if o["alt_psx"]:
                            ps_x = psum_xt.tile(
                                [P, C], F32, tag=f"psx{kc % 2}", bufs=1,
                                name="ps_x",
                            )
                        else:
                            ps_x = psum_xt.tile([P, C], F32, tag="psx",
                                                name="ps_x")
                        for co in range(CO):
                            nc.tensor.transpose(
                                ps_x[:, co * P:(co + 1) * P], X[:, co, ks], ident
                            )
                        xt_k = xtp.tile([P, C], MMDT, tag="xt", name="xt_k")
                        nc.scalar.copy(xt_k[:, :], ps_x[:, :])
                        return xt_k

                    def emit_mm1(kc, xt_k):
                        for ic in range(CO):
                            nc.tensor.matmul(
                                E[:, ic, ic * P:],
                                xt_k[:, ic * P:(ic + 1) * P],
                                xt_k[:, ic * P:],
                                start=(kc == 0),
                                stop=(kc == KC - 1),
                            )

                    if o["sw_pipe1"]:
                        pend = {}
                        for kc in range(KC):
                            pend[kc] = emit_trans(kc)
                            if kc >= 1:
                                emit_mm1(kc - 1, pend.pop(kc - 1))
                        emit_mm1(KC - 1, pend.pop(KC - 1))
                    else:
                        for kc in range(KC):
                            emit_mm1(kc, emit_trans(kc))
                    # mirror E[jc, ic] = E[ic, jc].T for ic < jc
                    for jc in range(1, CO):
                        for ic in range(jc):
                            stg = xtp.tile([P, P], F32, tag="mirror_stage")
                            nc.scalar.copy(
                                stg[:, :], E[:, ic, jc * P:(jc + 1) * P]
                            )
                            nc.tensor.matmul(
                                E[:, jc, ic * P:(ic + 1) * P],
                                stg[:, :],
                                ident,
                                is_transpose=True,
                                skip_group_check=True,
                            )

                    # ---- softmax: t = exp(mn - E), Z row-sum fused ----
                    mn = stats.tile([P, CO], F32, tag="mn")
                    zs = stats.tile([P, CO], F32, tag="zs")
                    rg = stats.tile([P, CO], F32, tag="rg")
                    tS = tpool.tile([P, CO, C], F32, tag="t")
                    for ic in range(CO):
                        nc.vector.tensor_reduce(
                            mn[:, ic:ic + 1], E[:, ic, :], AX.X, ALU.min
                        )
                    for ic in range(CO):
                        nc.scalar.activation(
                            tS[:, ic, :], E[:, ic, :], AF.Exp,
                            bias=mn[:, ic:ic + 1], scale=-1.0,
                            accum_out=zs[:, ic:ic + 1],
                        )
                    nc.vector.reciprocal(rg[:, :], zs[:, :])
                    nc.vector.tensor_scalar_mul(rg[:, :], rg[:, :], g_col[:, :1])

                    # ---- tT[j, i] = t[i, j] ----
                    tT = ttpool.tile([P, CO, C], BF16, tag="tT")
                    if o["tt_sym"]:
                        # tT[j,i] = exp(mn[i] - E[j,i]) (E symmetric).
                        # mn ([P, CO], i per-partition) -> mn_vec [1, C]
                        # (i on free) -> broadcast along partitions.
                        ps_mn = psum_xt.tile([CO, P], F32, tag="psx",
                                             name="ps_mn")
                        nc.tensor.transpose(ps_mn[:, :], mn[:, :], ident)
                        mnT = x16pool.tile([CO, P], F32, tag="mnT", bufs=1,
                                           name="mnT")
                        nc.vector.tensor_copy(mnT[:, :], ps_mn[:, :])
                        mnv = x16pool.tile([1, C], F32, tag="mnv", bufs=1,
                                           name="mnv")
                        nc.sync.dma_start(mnv[:, :], mnT[:, :])
                        mnb = x16pool.tile([P, C], F32, tag="mnb", bufs=1,
                                           name="mnb")
                        nc.gpsimd.partition_broadcast(mnb[:, :], mnv[:1, :])
                        for jc in range(CO):
                            tmp = opool.tile([P, C], F32, tag="o", name="tmp")
                            nc.vector.tensor_sub(
                                tmp[:, :], mnb[:, :], E[:, jc, :]
                            )
                            nc.scalar.activation(
                                tT[:, jc, :], tmp[:, :], AF.Exp,
                            )
                    else:
                        for jc in range(CO):
                            if o["alt_acc"]:
                                ps_t = psum_acc.tile(
                                    [P, C], F32, tag=f"acc{jc % 2}", bufs=1
                                )
                            else:
                                ps_t = psum_acc.tile([P, C], F32, tag="acc")
                            for ic in range(CO):
                                nc.tensor.transpose(
                                    ps_t[:, ic * P:(ic + 1) * P],
                                    tS[:, ic, jc * P:(jc + 1) * P],
                                    ident,
                                )
                            nc.scalar.copy(tT[:, jc, :], ps_t[:, :])

                    # ---- out = att @ x, scaled by gamma/Z + residual ----
                    for ic in range(CO):
                        for nf in range(NF):
                            ns = slice(nf * 512, (nf + 1) * 512)
                            if o["alt_acc"]:
                                ps2 = psum_acc.tile(
                                    [P, C], F32,
                                    tag=f"acc{(ic * NF + nf) % 2}", bufs=1,
                                )
                            else:
                                ps2 = psum_acc.tile([P, C], F32, tag="acc")
                            for jc in range(CO):
                                nc.tensor.matmul(
                                    ps2[:, :512],
                                    tT[:, jc, ic * P:(ic + 1) * P],
                                    X16[:, jc, ns],
                                    start=(jc == 0),
                                    stop=(jc == CO - 1),
                                )
                            ot = opool.tile([P, 512], F32, tag="o")
                            if o["evac_split"]:
                                nc.scalar.activation(
                                    ot[:, :], ps2[:, :512], AF.Copy,
                                    bias=0.0, scale=rg[:, ic:ic + 1],
                                )
                                nc.vector.tensor_add(
                                    ot[:, :], ot[:, :], X[:, ic, ns]
                                )
                            else:
                                nc.vector.scalar_tensor_tensor(
                                    ot[:, :], ps2[:, :512], rg[:, ic:ic + 1],
                                    X[:, ic, ns],
                                    op0=ALU.mult, op1=ALU.add,
                                )
                            nc.sync.dma_start(y_b[:, ic, ns], ot[:, :])

            if o["timing_io"]:
                # tiny external output depending on the body's last writes
                ysb = stats.tile([1, 1], F32, tag="ysb")
                nc.sync.dma_start(
                    ysb[:1, :1], y_out[Bs - 1, C - 1:C, N - 1:N]
                )
                nc.sync.dma_start(yy_out[:1, :1], ysb[:1, :1])

    nc.compile()
    return nc


def get_nc(Bs=4, C=512, N=4096, use_f32r=False, reps=1, **opts):
    key = (Bs, C, N, use_f32r, reps, tuple(sorted(opts.items())))
    if key not in _CACHE:
        _CACHE[key] = _build(Bs, C, N, use_f32r, reps, **opts)
    return _CACHE[key]


def kernel(x, gamma):
    """Full inputs in, full output out. x [32, 512, 4096] f32, gamma [1] f32."""
    from concourse.bass_utils import run_bass_kernel_spmd

    x = np.ascontiguousarray(np.asarray(x, dtype=np.float32))
    gamma = np.ascontiguousarray(np.asarray(gamma, dtype=np.float32))
    B, C, N = x.shape
    n_cores = 8
    assert B % n_cores == 0
    Bs = B // n_cores

    nc = get_nc(Bs, C, N)
    in_maps = [
        {"x": x[i * Bs:(i + 1) * Bs], "gamma": gamma} for i in range(n_cores)
    ]
    res = run_bass_kernel_spmd(nc, in_maps, core_ids=list(range(n_cores)))
    return np.concatenate([r["y"] for r in res.results], axis=0)

